# revision 5
# baseline (speedup 1.0000x reference)
"""LISSOM cortex layer forward pass on 8 Trainium2 NeuronCores.

Math (reference):
    afferent = clamp(x @ Wr, 0, 1)                      # [B, N]
    exc      = clamp(afferent @ We, 0, 1)               # [B, N]
    inh      = clamp(afferent @ Wi, 0, 1)               # [B, N]
    out      = clamp(afferent + 0.2*exc - 0.4*inh, 0, 1)

Sharding: the three [N, N] weight matrices are column-sharded across 8
cores ([N, N/8] per core). x is replicated (fed pre-transposed as
xT [N, B] so it can be the stationary matmul operand directly). Each
core computes its afferent column slice [B, N/8], clamps it, transposes
it on the PE to [N/8, B], and AllGathers to the full [N, B] transposed
afferent, which is exactly the stationary-operand layout the two
lateral matmuls need. The final combine stays in [B, N/8] layout and
each core writes its output column slice; the host concatenates.

Weights (and the matmul inputs) are stored as bf16: PSUM accumulation
stays fp32 and with K=9216 the rounding error averages out to ~1e-4
relative, while halving the HBM traffic that dominates this kernel.
The afferent used in the final combine is kept in fp32.
"""

import sys

if "/opt/trn_rl_repo" not in sys.path:
    sys.path.insert(0, "/opt/trn_rl_repo")

import ml_dtypes
import numpy as np

import concourse.bass as bass
import concourse.bacc as bacc
import concourse.mybir as mybir
import concourse.tile as tile
from concourse.bass_utils import run_bass_kernel_spmd

B = 32            # batch
N = 9216          # neurons
CORES = 8
S = N // CORES    # 1152 columns per core
KP = 128          # contraction tile (partition dim)
KC = N // KP      # 72 k-chunks
NS = 384          # matmul free-dim slice (3 per core slice, 1 PSUM bank each)
NJ = S // NS      # 3 n-slices

F32 = mybir.dt.float32

USE_BF16 = True


def build_nc(use_bf16=USE_BF16):
    DT = mybir.dt.bfloat16 if use_bf16 else F32
    np_dt = ml_dtypes.bfloat16 if use_bf16 else np.float32

    nc = bacc.Bacc("TRN2", num_devices=CORES)

    xT_d = nc.dram_tensor("xT", [N, B], DT, kind="ExternalInput")
    wr_d = nc.dram_tensor("wr", [N, S], DT, kind="ExternalInput")
    we_d = nc.dram_tensor("we", [N, S], DT, kind="ExternalInput")
    wi_d = nc.dram_tensor("wi", [N, S], DT, kind="ExternalInput")
    out_d = nc.dram_tensor("out", [B, S], F32, kind="ExternalOutput")
    ident_d = nc.inline_tensor(np.eye(32, dtype=np_dt), name="ident32")

    rg = [list(range(CORES))]

    with tile.TileContext(nc) as tc:
        with (
            tc.tile_pool(name="persist", bufs=1) as persist,
            tc.tile_pool(name="wrp", bufs=3) as wrp,
            tc.tile_pool(name="wep", bufs=5) as wep,
            tc.tile_pool(name="wip", bufs=5) as wip,
            tc.tile_pool(name="ps", bufs=1, space="PSUM") as ps,
            tc.tile_pool(name="dram", bufs=1, space="DRAM") as dram,
        ):
            # --- replicated x, pre-transposed: [N, B] -> SBUF [128, KC*B]
            xT_sb = persist.tile([KP, KC * B], DT)
            nc.sync.dma_start(
                xT_sb[:].rearrange("p (k b) -> p k b", b=B),
                xT_d[:].rearrange("(k p) b -> k p b", p=KP).transpose([1, 0, 2]),
            )
            ident_sb = persist.tile([32, 32], DT)
            nc.sync.dma_start(ident_sb[:], ident_d[:])

            # --- phase 1: afferent[B, S] = clamp(x @ Wr_slice) ------------
            aff_ps = [
                ps.tile([B, NS], F32, name=f"aff_ps{j}", tag="acc", bufs=6)
                for j in range(NJ)
            ]
            for k in range(KC):
                wr_t = wrp.tile([KP, S], DT, name="wr_t", tag="wr")
                nc.sync.dma_start(wr_t[:], wr_d[k * KP : (k + 1) * KP, :])
                lhsT = xT_sb[:, k * B : (k + 1) * B]
                for j in range(NJ):
                    nc.tensor.matmul(
                        aff_ps[j][:],
                        lhsT,
                        wr_t[:, j * NS : (j + 1) * NS],
                        start=(k == 0),
                        stop=(k == KC - 1),
                    )

            # fp32 afferent for the final combine; DT copy for the matmul path
            aff_sb = persist.tile([B, S], F32)
            aff16_sb = persist.tile([B, S], DT)
            for j in range(NJ):
                js = slice(j * NS, (j + 1) * NS)
                nc.vector.tensor_scalar(
                    aff_sb[:, js], aff_ps[j][:], 0.0, 1.0,
                    mybir.AluOpType.max, mybir.AluOpType.min,
                )
                nc.vector.tensor_scalar(
                    aff16_sb[:, js], aff_ps[j][:], 0.0, 1.0,
                    mybir.AluOpType.max, mybir.AluOpType.min,
                )

            # --- transpose local afferent slice to [S, B] on the PE -------
            affT_sb = persist.tile([KP, (S // KP) * B], DT)  # [128, 9*32]
            for m in range(S // KP):
                tp_ps = ps.tile([KP, B], DT, name="tp_ps", tag="tp", bufs=2)
                nc.tensor.transpose(
                    tp_ps[:], aff16_sb[:, m * KP : (m + 1) * KP], ident_sb[:]
                )
                nc.vector.tensor_copy(affT_sb[:, m * B : (m + 1) * B], tp_ps[:])

            # --- AllGather afferent^T across the 8 cores ------------------
            ag_in = dram.tile([S, B], DT, name="ag_in")
            ag_out = dram.tile([N, B], DT, name="ag_out", addr_space="Shared")
            nc.sync.dma_start(
                ag_in[:].rearrange("(m p) b -> m p b", p=KP).transpose([1, 0, 2]),
                affT_sb[:].rearrange("p (m b) -> p m b", b=B),
            )
            nc.gpsimd.collective_compute(
                "AllGather",
                mybir.AluOpType.bypass,
                replica_groups=rg,
                ins=[ag_in.opt()],
                outs=[ag_out.opt()],
            )
            affTg_sb = persist.tile([KP, KC * B], DT)
            nc.sync.dma_start(
                affTg_sb[:].rearrange("p (k b) -> p k b", b=B),
                ag_out[:].rearrange("(k p) b -> k p b", p=KP).transpose([1, 0, 2]),
            )

            # --- phase 2: exc/inh column slices ---------------------------
            exc_ps = [
                ps.tile([B, NS], F32, name=f"exc_ps{j}", tag="acc", bufs=6)
                for j in range(NJ)
            ]
            inh_ps = [
                ps.tile([B, NS], F32, name=f"inh_ps{j}", tag="acc", bufs=6)
                for j in range(NJ)
            ]
            for k in range(KC):
                we_t = wep.tile([KP, S], DT, name="we_t", tag="we")
                wi_t = wip.tile([KP, S], DT, name="wi_t", tag="wi")
                nc.sync.dma_start(we_t[:], we_d[k * KP : (k + 1) * KP, :])
                nc.sync.dma_start(wi_t[:], wi_d[k * KP : (k + 1) * KP, :])
                lhsT = affTg_sb[:, k * B : (k + 1) * B]
                for j in range(NJ):
                    nc.tensor.matmul(
                        exc_ps[j][:],
                        lhsT,
                        we_t[:, j * NS : (j + 1) * NS],
                        start=(k == 0),
                        stop=(k == KC - 1),
                    )
                for j in range(NJ):
                    nc.tensor.matmul(
                        inh_ps[j][:],
                        lhsT,
                        wi_t[:, j * NS : (j + 1) * NS],
                        start=(k == 0),
                        stop=(k == KC - 1),
                    )

            # --- combine: clamp(aff + 0.2*clamp(exc) - 0.4*clamp(inh)) ----
            out_sb = persist.tile([B, S], F32)
            for j in range(NJ):
                js = slice(j * NS, (j + 1) * NS)
                exc_c = persist.tile([B, NS], F32, name=f"exc_c{j}")
                inh_c = persist.tile([B, NS], F32, name=f"inh_c{j}")
                nc.vector.tensor_scalar(
                    exc_c[:], exc_ps[j][:], 0.0, 1.0,
                    mybir.AluOpType.max, mybir.AluOpType.min,
                )
                nc.vector.tensor_scalar(
                    inh_c[:], inh_ps[j][:], 0.0, 1.0,
                    mybir.AluOpType.max, mybir.AluOpType.min,
                )
                t0 = persist.tile([B, NS], F32, name=f"t0_{j}")
                nc.vector.scalar_tensor_tensor(
                    t0[:], exc_c[:], 0.2, aff_sb[:, js],
                    mybir.AluOpType.mult, mybir.AluOpType.add,
                )
                t1 = persist.tile([B, NS], F32, name=f"t1_{j}")
                nc.vector.scalar_tensor_tensor(
                    t1[:], inh_c[:], -0.4, t0[:],
                    mybir.AluOpType.mult, mybir.AluOpType.add,
                )
                nc.vector.tensor_scalar(
                    out_sb[:, js], t1[:], 0.0, 1.0,
                    mybir.AluOpType.max, mybir.AluOpType.min,
                )
            nc.sync.dma_start(out_d[:], out_sb[:])

    nc.compile()
    return nc


_NC = None


def _get_nc():
    global _NC
    if _NC is None:
        _NC = build_nc()
    return _NC


def _run(x, retina_weights, excitatory_weights, inhibitory_weights, trace=False):
    np_dt = ml_dtypes.bfloat16 if USE_BF16 else np.float32
    x = np.asarray(x, dtype=np.float32)
    xT = np.ascontiguousarray(x.T).astype(np_dt)
    wr = np.asarray(retina_weights, dtype=np.float32)
    we = np.asarray(excitatory_weights, dtype=np.float32)
    wi = np.asarray(inhibitory_weights, dtype=np.float32)

    in_maps = []
    for c in range(CORES):
        sl = slice(c * S, (c + 1) * S)
        in_maps.append(
            {
                "xT": xT,
                "wr": np.ascontiguousarray(wr[:, sl]).astype(np_dt),
                "we": np.ascontiguousarray(we[:, sl]).astype(np_dt),
                "wi": np.ascontiguousarray(wi[:, sl]).astype(np_dt),
            }
        )

    res = run_bass_kernel_spmd(
        _get_nc(), in_maps, core_ids=list(range(CORES)), trace=trace
    )
    out = np.concatenate([res.results[c]["out"] for c in range(CORES)], axis=1)
    return np.ascontiguousarray(out, dtype=np.float32), res


def kernel(x, retina_weights, excitatory_weights, inhibitory_weights):
    out, _ = _run(x, retina_weights, excitatory_weights, inhibitory_weights)
    return out


# revision 8
# speedup vs baseline: 1.0059x; 1.0059x over previous
"""LISSOM cortex layer forward pass on 8 Trainium2 NeuronCores.

Math (reference):
    afferent = clamp(x @ Wr, 0, 1)                      # [B, N]
    exc      = clamp(afferent @ We, 0, 1)               # [B, N]
    inh      = clamp(afferent @ Wi, 0, 1)               # [B, N]
    out      = clamp(afferent + 0.2*exc - 0.4*inh, 0, 1)

Sharding: the three [N, N] weight matrices are column-sharded across 8
cores ([N, N/8] per core). x is replicated (fed pre-transposed as
xT [N, B] so it can be the stationary matmul operand directly). Each
core computes its afferent column slice [B, N/8], clamps it, transposes
it on the PE to [N/8, B], and AllGathers to the full [N, B] transposed
afferent, which is exactly the stationary-operand layout the two
lateral matmuls need. The final combine stays in [B, N/8] layout and
each core writes its output column slice; the host concatenates.

Weights (and the matmul inputs) are stored as bf16: PSUM accumulation
stays fp32 and with K=9216 the rounding error averages out to ~1e-4
relative, while halving the HBM traffic that dominates this kernel.
The afferent used in the final combine is kept in fp32.
"""

import sys

if "/opt/trn_rl_repo" not in sys.path:
    sys.path.insert(0, "/opt/trn_rl_repo")

import ml_dtypes
import numpy as np

import concourse.bass as bass
import concourse.bacc as bacc
import concourse.mybir as mybir
import concourse.tile as tile
from concourse.bass_utils import run_bass_kernel_spmd

B = 32            # batch
N = 9216          # neurons
CORES = 8
S = N // CORES    # 1152 columns per core
KP = 128          # contraction tile (partition dim)
KC = N // KP      # 72 k-chunks
NS = 384          # matmul free-dim slice (3 per core slice, 1 PSUM bank each)
NJ = S // NS      # 3 n-slices

F32 = mybir.dt.float32
BF16 = mybir.dt.bfloat16
E8 = mybir.dt.float8e4  # e4m3


def build_nc():
    DT = BF16
    np_dt = ml_dtypes.bfloat16

    nc = bacc.Bacc("TRN2", num_devices=CORES)

    xT_d = nc.dram_tensor("xT", [KP, KC * B], DT, kind="ExternalInput")
    wr_d = nc.dram_tensor("wr", [N, S], E8, kind="ExternalInput")
    we_d = nc.dram_tensor("we", [N, S], DT, kind="ExternalInput")
    wi_d = nc.dram_tensor("wi", [N, S], E8, kind="ExternalInput")
    # per-partition broadcast of the fp8 descale factors [1/sr, 1/si]
    scales_d = nc.dram_tensor("scales", [B, 2], F32, kind="ExternalInput")
    out_d = nc.dram_tensor("out", [B, S], F32, kind="ExternalOutput")
    ident_d = nc.inline_tensor(np.eye(32, dtype=np_dt), name="ident32")

    rg = [list(range(CORES))]

    with tile.TileContext(nc) as tc:
        with (
            tc.tile_pool(name="persist", bufs=1) as persist,
            tc.tile_pool(name="wrp", bufs=8) as wrp,
            tc.tile_pool(name="wep", bufs=12) as wep,
            tc.tile_pool(name="wip", bufs=12) as wip,
            tc.tile_pool(name="ps", bufs=1, space="PSUM") as ps,
            tc.tile_pool(name="dram", bufs=1, space="DRAM") as dram,
        ):
            # --- replicated x, pre-transposed: [N, B] -> SBUF [128, KC*B]
            xT_sb = persist.tile([KP, KC * B], DT)
            nc.sync.dma_start(xT_sb[:], xT_d[:])
            ident_sb = persist.tile([32, 32], DT)
            nc.sync.dma_start(ident_sb[:], ident_d[:])
            scales_sb = persist.tile([B, 2], F32)
            nc.sync.dma_start(scales_sb[:], scales_d[:])

            # --- phase 1: afferent[B, S] = clamp(x @ Wr_slice) ------------
            aff_ps = [
                ps.tile([B, NS], F32, name=f"aff_ps{j}", tag="acc", bufs=6)
                for j in range(NJ)
            ]
            for k in range(KC):
                wr_t = wrp.tile([KP, S], E8, name="wr_t", tag="wr")
                nc.sync.dma_start(wr_t[:], wr_d[k * KP : (k + 1) * KP, :])
                lhsT = xT_sb[:, k * B : (k + 1) * B]
                for j in range(NJ):
                    nc.tensor.matmul(
                        aff_ps[j][:],
                        lhsT,
                        wr_t[:, j * NS : (j + 1) * NS],
                        start=(k == 0),
                        stop=(k == KC - 1),
                    )

            # fp32 afferent for the final combine; DT copy for the matmul path
            aff_sb = persist.tile([B, S], F32)
            aff16_sb = persist.tile([B, S], DT)
            for j in range(NJ):
                js = slice(j * NS, (j + 1) * NS)
                affr = persist.tile([B, NS], F32, name=f"affr{j}")
                nc.vector.tensor_scalar(
                    affr[:], aff_ps[j][:], scales_sb[:, 0:1], 0.0,
                    mybir.AluOpType.mult, mybir.AluOpType.max,
                )
                nc.vector.tensor_scalar_min(aff_sb[:, js], affr[:], 1.0)
                nc.vector.tensor_scalar_min(aff16_sb[:, js], affr[:], 1.0)

            # --- transpose local afferent slice to [S, B] on the PE -------
            affT_sb = persist.tile([KP, (S // KP) * B], DT)  # [128, 9*32]
            for m in range(S // KP):
                tp_ps = ps.tile([KP, B], DT, name="tp_ps", tag="tp", bufs=2)
                nc.tensor.transpose(
                    tp_ps[:], aff16_sb[:, m * KP : (m + 1) * KP], ident_sb[:]
                )
                nc.vector.tensor_copy(affT_sb[:, m * B : (m + 1) * B], tp_ps[:])

            # --- AllGather afferent^T across the 8 cores ------------------
            ag_in = dram.tile([S, B], DT, name="ag_in")
            ag_out = dram.tile([N, B], DT, name="ag_out", addr_space="Shared")
            nc.sync.dma_start(
                ag_in[:].rearrange("(m p) b -> m p b", p=KP).transpose([1, 0, 2]),
                affT_sb[:].rearrange("p (m b) -> p m b", b=B),
            )
            nc.gpsimd.collective_compute(
                "AllGather",
                mybir.AluOpType.bypass,
                replica_groups=rg,
                ins=[ag_in.opt()],
                outs=[ag_out.opt()],
            )
            affTg_sb = persist.tile([KP, KC * B], DT)
            kc_r = KC // CORES  # 9 k-chunks per rank block
            for r in range(CORES):
                nc.sync.dma_start(
                    affTg_sb[:, r * kc_r * B : (r + 1) * kc_r * B].rearrange(
                        "p (k b) -> p k b", b=B
                    ),
                    ag_out[r * kc_r * KP : (r + 1) * kc_r * KP, :]
                    .rearrange("(k p) b -> k p b", p=KP)
                    .transpose([1, 0, 2]),
                )

            # --- phase 2: exc/inh column slices ---------------------------
            exc_ps = [
                ps.tile([B, NS], F32, name=f"exc_ps{j}", tag="acc", bufs=6)
                for j in range(NJ)
            ]
            inh_ps = [
                ps.tile([B, NS], F32, name=f"inh_ps{j}", tag="acc", bufs=6)
                for j in range(NJ)
            ]
            for k in range(KC):
                we_t = wep.tile([KP, S], DT, name="we_t", tag="we")
                wi_t = wip.tile([KP, S], E8, name="wi_t", tag="wi")
                nc.sync.dma_start(we_t[:], we_d[k * KP : (k + 1) * KP, :])
                nc.sync.dma_start(wi_t[:], wi_d[k * KP : (k + 1) * KP, :])
                lhsT = affTg_sb[:, k * B : (k + 1) * B]
                for j in range(NJ):
                    nc.tensor.matmul(
                        exc_ps[j][:],
                        lhsT,
                        we_t[:, j * NS : (j + 1) * NS],
                        start=(k == 0),
                        stop=(k == KC - 1),
                    )
                for j in range(NJ):
                    nc.tensor.matmul(
                        inh_ps[j][:],
                        lhsT,
                        wi_t[:, j * NS : (j + 1) * NS],
                        start=(k == 0),
                        stop=(k == KC - 1),
                    )

            # --- combine: clamp(aff + 0.2*clamp(exc) - 0.4*clamp(inh)) ----
            out_sb = persist.tile([B, S], F32)
            for j in range(NJ):
                js = slice(j * NS, (j + 1) * NS)
                exc_c = persist.tile([B, NS], F32, name=f"exc_c{j}")
                inh_c = persist.tile([B, NS], F32, name=f"inh_c{j}")
                nc.vector.tensor_scalar(
                    exc_c[:], exc_ps[j][:], 0.0, 1.0,
                    mybir.AluOpType.max, mybir.AluOpType.min,
                )
                inhr = persist.tile([B, NS], F32, name=f"inhr{j}")
                nc.vector.tensor_scalar(
                    inhr[:], inh_ps[j][:], scales_sb[:, 1:2], 0.0,
                    mybir.AluOpType.mult, mybir.AluOpType.max,
                )
                nc.vector.tensor_scalar_min(inh_c[:], inhr[:], 1.0)
                t0 = persist.tile([B, NS], F32, name=f"t0_{j}")
                nc.vector.scalar_tensor_tensor(
                    t0[:], exc_c[:], 0.2, aff_sb[:, js],
                    mybir.AluOpType.mult, mybir.AluOpType.add,
                )
                t1 = persist.tile([B, NS], F32, name=f"t1_{j}")
                nc.vector.scalar_tensor_tensor(
                    t1[:], inh_c[:], -0.4, t0[:],
                    mybir.AluOpType.mult, mybir.AluOpType.add,
                )
                nc.vector.tensor_scalar(
                    out_sb[:, js], t1[:], 0.0, 1.0,
                    mybir.AluOpType.max, mybir.AluOpType.min,
                )
            nc.sync.dma_start(out_d[:], out_sb[:])

    nc.compile()
    return nc


_NC = None


def _get_nc():
    global _NC
    if _NC is None:
        _NC = build_nc()
    return _NC


def make_in_maps(x, retina_weights, excitatory_weights, inhibitory_weights):
    import concourse.mybir as _mb

    np_bf = ml_dtypes.bfloat16
    np_e8 = _mb.dt.np(E8)
    x = np.asarray(x, dtype=np.float32)
    xT = np.ascontiguousarray(
        x.T.reshape(KC, KP, B).transpose(1, 0, 2).reshape(KP, KC * B)
    ).astype(np_bf)
    wr = np.asarray(retina_weights, dtype=np.float32)
    we = np.asarray(excitatory_weights, dtype=np.float32)
    wi = np.asarray(inhibitory_weights, dtype=np.float32)
    sr = 192.0 / max(float(np.abs(wr).max()), 1e-30)
    si = 192.0 / max(float(np.abs(wi).max()), 1e-30)
    scales = np.tile(
        np.array([[1.0 / sr, 1.0 / si]], dtype=np.float32), (B, 1)
    )

    in_maps = []
    for c in range(CORES):
        sl = slice(c * S, (c + 1) * S)
        in_maps.append(
            {
                "xT": xT,
                "wr": (np.ascontiguousarray(wr[:, sl]) * sr).astype(np_e8),
                "we": np.ascontiguousarray(we[:, sl]).astype(np_bf),
                "wi": (np.ascontiguousarray(wi[:, sl]) * si).astype(np_e8),
                "scales": scales,
            }
        )
    return in_maps


def _run(x, retina_weights, excitatory_weights, inhibitory_weights, trace=False):
    in_maps = make_in_maps(
        x, retina_weights, excitatory_weights, inhibitory_weights
    )

    res = run_bass_kernel_spmd(
        _get_nc(), in_maps, core_ids=list(range(CORES)), trace=trace
    )
    out = np.concatenate([res.results[c]["out"] for c in range(CORES)], axis=1)
    return np.ascontiguousarray(out, dtype=np.float32), res


def kernel(x, retina_weights, excitatory_weights, inhibitory_weights):
    out, _ = _run(x, retina_weights, excitatory_weights, inhibitory_weights)
    return out


# revision 14
# speedup vs baseline: 311.9204x; 310.1023x over previous
"""LISSOM cortex layer forward pass on 8 Trainium2 NeuronCores.

Math (reference):
    afferent = clamp(x @ Wr, 0, 1)                      # [B, N]
    exc      = clamp(afferent @ We, 0, 1)               # [B, N]
    inh      = clamp(afferent @ Wi, 0, 1)               # [B, N]
    out      = clamp(afferent + 0.2*exc - 0.4*inh, 0, 1)

Sharding: the three [N, N] weight matrices are column-sharded across 8
cores ([N, N/8] per core). x is replicated (fed pre-transposed as
xT [N, B] so it can be the stationary matmul operand directly). Each
core computes its afferent column slice [B, N/8], clamps it, transposes
it on the PE to [N/8, B], and AllGathers to the full [N, B] transposed
afferent, which is exactly the stationary-operand layout the two
lateral matmuls need. The final combine stays in [B, N/8] layout and
each core writes its output column slice; the host concatenates.

Weights (and the matmul inputs) are stored as bf16: PSUM accumulation
stays fp32 and with K=9216 the rounding error averages out to ~1e-4
relative, while halving the HBM traffic that dominates this kernel.
The afferent used in the final combine is kept in fp32.
"""

import sys

if "/opt/trn_rl_repo" not in sys.path:
    sys.path.insert(0, "/opt/trn_rl_repo")

import ml_dtypes
import numpy as np

import concourse.bass as bass
import concourse.bacc as bacc
import concourse.mybir as mybir
import concourse.tile as tile
from concourse.tile import add_dep_helper
from concourse.bass_utils import run_bass_kernel_spmd

B = 32            # batch
N = 9216          # neurons
CORES = 8
S = N // CORES    # 1152 columns per core
KP = 128          # contraction tile (partition dim)
KC = N // KP      # 72 k-chunks
NS = 384          # matmul free-dim slice (3 per core slice, 1 PSUM bank each)
NJ = S // NS      # 3 n-slices

F32 = mybir.dt.float32
BF16 = mybir.dt.bfloat16
E8 = mybir.dt.float8e4  # e4m3


def build_nc():
    DT = BF16
    np_dt = ml_dtypes.bfloat16

    nc = bacc.Bacc("TRN2", num_devices=CORES)

    xT_d = nc.dram_tensor("xT", [KP, KC * B], DT, kind="ExternalInput")
    wr_d = nc.dram_tensor("wr", [N, S], E8, kind="ExternalInput")
    we_d = nc.dram_tensor("we", [N, S], DT, kind="ExternalInput")
    wi_d = nc.dram_tensor("wi", [N, S], E8, kind="ExternalInput")
    # per-partition broadcast of the fp8 descale factors [1/sr, 1/si]
    scales_d = nc.dram_tensor("scales", [B, 2], F32, kind="ExternalInput")
    out_d = nc.dram_tensor("out", [B, S], F32, kind="ExternalOutput")
    ident_d = nc.inline_tensor(np.eye(32, dtype=np_dt), name="ident32")

    rg = [list(range(CORES))]
    G = 4           # PE column-group packing: 4 k-chunks run concurrently
    KB = KC // G    # 18 outer iterations, one 4-chunk weight DMA each

    def packed_matmul_chain(w_d, w_pool, w_dt, lhs_sb, ps_tiles, dep_to=None):
        """KB iterations; iteration kb DMAs chunks [kb*G, kb*G+G) and issues
        G col-group matmuls per j-slice, accumulating into partition group
        32*t of ps_tiles[j]. Returns the last DMA instruction."""
        last_dma = None
        for kb in range(KB):
            w_t = w_pool.tile([KP, G * S], w_dt, name="w_t", tag=w_pool.name)
            src_sl = slice(kb * G * KP, (kb + 1) * G * KP)
            last_dma = nc.sync.dma_start(
                w_t[:].rearrange("p (t s) -> p t s", s=S),
                w_d[src_sl, :].rearrange("(t p) s -> t p s", p=KP)
                .transpose([1, 0, 2]),
            )
            if dep_to is not None:
                add_dep_helper(
                    last_dma.ins, dep_to.ins, sync=True,
                    reason="prefetch after critical wr stream",
                )
            for t in range(G):
                k = kb * G + t
                lhsT = lhs_sb[:, k * B : (k + 1) * B]
                for j in range(NJ):
                    nc.tensor.matmul(
                        ps_tiles[j][32 * t : 32 * (t + 1), :],
                        lhsT,
                        w_t[:, t * S + j * NS : t * S + (j + 1) * NS],
                        start=(kb == 0),
                        stop=(kb == KB - 1),
                        tile_position=(0, 32 * t),
                    )
        return last_dma

    def group_reduce(pool, ps, j, name):
        """Sum the 4 col-group partials of ps -> [B, NS] fp32 in SBUF.
        DVE may read at most one PSUM operand per instruction, so chain."""
        v0 = pool.tile([B, NS], F32, name=f"{name}v0_{j}")
        v1 = pool.tile([B, NS], F32, name=f"{name}v1_{j}")
        v2 = pool.tile([B, NS], F32, name=f"{name}v2_{j}")
        v3 = pool.tile([B, NS], F32, name=f"{name}v3_{j}")
        nc.vector.tensor_copy(v0[:], ps[0:32, :])
        nc.vector.scalar_tensor_tensor(
            v1[:], v0[:], 1.0, ps[32:64, :],
            mybir.AluOpType.mult, mybir.AluOpType.add,
        )
        nc.vector.scalar_tensor_tensor(
            v2[:], v1[:], 1.0, ps[64:96, :],
            mybir.AluOpType.mult, mybir.AluOpType.add,
        )
        nc.vector.scalar_tensor_tensor(
            v3[:], v2[:], 1.0, ps[96:128, :],
            mybir.AluOpType.mult, mybir.AluOpType.add,
        )
        return v3

    with tile.TileContext(nc) as tc:
        with (
            tc.tile_pool(name="persist", bufs=1) as persist,
            tc.tile_pool(name="wr", bufs=6) as wrp,
            tc.tile_pool(name="we", bufs=5) as wep,
            tc.tile_pool(name="wi", bufs=5) as wip,
            tc.tile_pool(name="ps", bufs=1, space="PSUM") as ps,
            tc.tile_pool(name="dram", bufs=1, space="DRAM") as dram,
        ):
            # --- replicated x, pre-permuted to the SBUF layout -------------
            xT_sb = persist.tile([KP, KC * B], DT)
            nc.sync.dma_start(xT_sb[:], xT_d[:])
            ident_sb = persist.tile([32, 32], DT)
            nc.sync.dma_start(ident_sb[:], ident_d[:])
            scales_sb = persist.tile([B, 2], F32)
            nc.sync.dma_start(scales_sb[:], scales_d[:])

            # --- phase 1: afferent[B, S] = clamp(x @ Wr_slice / sr) -------
            aff_ps = [
                ps.tile([KP, NS], F32, name=f"aff_ps{j}", tag="acc", bufs=6)
                for j in range(NJ)
            ]
            last_wr_dma = packed_matmul_chain(wr_d, wrp, E8, xT_sb, aff_ps)

            # fp32 afferent for the final combine; DT copy for the matmul path
            aff_sb = persist.tile([B, S], F32)
            aff16_sb = persist.tile([B, S], DT)
            for j in range(NJ):
                js = slice(j * NS, (j + 1) * NS)
                asum = group_reduce(persist, aff_ps[j], j, "a")
                affr = persist.tile([B, NS], F32, name=f"affr{j}")
                nc.vector.tensor_scalar(
                    affr[:], asum[:], scales_sb[:, 0:1], 0.0,
                    mybir.AluOpType.mult, mybir.AluOpType.max,
                )
                nc.vector.tensor_scalar_min(aff_sb[:, js], affr[:], 1.0)
                nc.vector.tensor_scalar_min(aff16_sb[:, js], affr[:], 1.0)

            # --- transpose local afferent slice to [S, B] on the PE -------
            affT_sb = persist.tile([KP, (S // KP) * B], DT)  # [128, 9*32]
            for m in range(S // KP):
                tp_ps = ps.tile([KP, B], DT, name="tp_ps", tag="tp", bufs=2)
                nc.tensor.transpose(
                    tp_ps[:], aff16_sb[:, m * KP : (m + 1) * KP], ident_sb[:]
                )
                nc.vector.tensor_copy(affT_sb[:, m * B : (m + 1) * B], tp_ps[:])

            # --- AllGather afferent^T across the 8 cores ------------------
            ag_in = dram.tile([S, B], DT, name="ag_in")
            ag_out = dram.tile([N, B], DT, name="ag_out", addr_space="Shared")
            nc.sync.dma_start(
                ag_in[:].rearrange("(m p) b -> m p b", p=KP).transpose([1, 0, 2]),
                affT_sb[:].rearrange("p (m b) -> p m b", b=B),
            )
            nc.gpsimd.collective_compute(
                "AllGather",
                mybir.AluOpType.bypass,
                replica_groups=rg,
                ins=[ag_in.opt()],
                outs=[ag_out.opt()],
            )
            affTg_sb = persist.tile([KP, KC * B], DT)
            kc_r = KC // CORES  # 9 k-chunks per rank block
            for r in range(CORES):
                nc.sync.dma_start(
                    affTg_sb[:, r * kc_r * B : (r + 1) * kc_r * B].rearrange(
                        "p (k b) -> p k b", b=B
                    ),
                    ag_out[r * kc_r * KP : (r + 1) * kc_r * KP, :]
                    .rearrange("(k p) b -> k p b", p=KP)
                    .transpose([1, 0, 2]),
                )

            # --- phase 2: exc/inh column slices ---------------------------
            exc_ps = [
                ps.tile([KP, NS], F32, name=f"exc_ps{j}", tag="acc", bufs=6)
                for j in range(NJ)
            ]
            inh_ps = [
                ps.tile([KP, NS], F32, name=f"inh_ps{j}", tag="acc", bufs=6)
                for j in range(NJ)
            ]
            packed_matmul_chain(we_d, wep, DT, affTg_sb, exc_ps, dep_to=last_wr_dma)
            packed_matmul_chain(wi_d, wip, E8, affTg_sb, inh_ps, dep_to=last_wr_dma)

            # --- combine: clamp(aff + 0.2*clamp(exc) - 0.4*clamp(inh/si)) -
            out_sb = persist.tile([B, S], F32)
            for j in range(NJ):
                js = slice(j * NS, (j + 1) * NS)
                esum = group_reduce(persist, exc_ps[j], j, "e")
                isum = group_reduce(persist, inh_ps[j], j, "i")
                exc_c = persist.tile([B, NS], F32, name=f"exc_c{j}")
                inh_c = persist.tile([B, NS], F32, name=f"inh_c{j}")
                nc.vector.tensor_scalar(
                    exc_c[:], esum[:], 0.0, 1.0,
                    mybir.AluOpType.max, mybir.AluOpType.min,
                )
                inhr = persist.tile([B, NS], F32, name=f"inhr{j}")
                nc.vector.tensor_scalar(
                    inhr[:], isum[:], scales_sb[:, 1:2], 0.0,
                    mybir.AluOpType.mult, mybir.AluOpType.max,
                )
                nc.vector.tensor_scalar_min(inh_c[:], inhr[:], 1.0)
                t0 = persist.tile([B, NS], F32, name=f"t0_{j}")
                nc.vector.scalar_tensor_tensor(
                    t0[:], exc_c[:], 0.2, aff_sb[:, js],
                    mybir.AluOpType.mult, mybir.AluOpType.add,
                )
                t1 = persist.tile([B, NS], F32, name=f"t1_{j}")
                nc.vector.scalar_tensor_tensor(
                    t1[:], inh_c[:], -0.4, t0[:],
                    mybir.AluOpType.mult, mybir.AluOpType.add,
                )
                nc.vector.tensor_scalar(
                    out_sb[:, js], t1[:], 0.0, 1.0,
                    mybir.AluOpType.max, mybir.AluOpType.min,
                )
            nc.sync.dma_start(out_d[:], out_sb[:])

    nc.compile()
    return nc


_NC = None


def _get_nc():
    global _NC
    if _NC is None:
        _NC = build_nc()
    return _NC


def make_in_maps(x, retina_weights, excitatory_weights, inhibitory_weights):
    import concourse.mybir as _mb

    np_bf = ml_dtypes.bfloat16
    np_e8 = _mb.dt.np(E8)
    x = np.asarray(x, dtype=np.float32)
    xT = np.ascontiguousarray(
        x.T.reshape(KC, KP, B).transpose(1, 0, 2).reshape(KP, KC * B)
    ).astype(np_bf)
    wr = np.asarray(retina_weights, dtype=np.float32)
    we = np.asarray(excitatory_weights, dtype=np.float32)
    wi = np.asarray(inhibitory_weights, dtype=np.float32)
    sr = 192.0 / max(float(np.abs(wr).max()), 1e-30)
    si = 192.0 / max(float(np.abs(wi).max()), 1e-30)
    scales = np.tile(
        np.array([[1.0 / sr, 1.0 / si]], dtype=np.float32), (B, 1)
    )

    in_maps = []
    for c in range(CORES):
        sl = slice(c * S, (c + 1) * S)
        in_maps.append(
            {
                "xT": xT,
                "wr": (np.ascontiguousarray(wr[:, sl]) * sr).astype(np_e8),
                "we": np.ascontiguousarray(we[:, sl]).astype(np_bf),
                "wi": (np.ascontiguousarray(wi[:, sl]) * si).astype(np_e8),
                "scales": scales,
            }
        )
    return in_maps


def _run(x, retina_weights, excitatory_weights, inhibitory_weights, trace=False):
    in_maps = make_in_maps(
        x, retina_weights, excitatory_weights, inhibitory_weights
    )

    res = run_bass_kernel_spmd(
        _get_nc(), in_maps, core_ids=list(range(CORES)), trace=trace
    )
    out = np.concatenate([res.results[c]["out"] for c in range(CORES)], axis=1)
    return np.ascontiguousarray(out, dtype=np.float32), res


def kernel(x, retina_weights, excitatory_weights, inhibitory_weights):
    out, _ = _run(x, retina_weights, excitatory_weights, inhibitory_weights)
    return out
